# revision 31
# baseline (speedup 1.0000x reference)
"""CenterLoss kernel for Trainium2 (8 NeuronCores, data-parallel over batch).

reference:  mean(clip(rowsum((x - labels @ centers)^2), 1e-12, 1e12))
labels are exact one-hot rows, so labels @ centers is an embedding gather:
    idx[b]  = max_index(labels[b, :])           (DVE max_index, query = 1.0)
    c[b]    = centers[idx[b], :]                (indirect DMA row gather)
    ps[b]   = rowsum((x[b] - c[b])^2)           (DVE sub, ACT square+f32 accum)

All three input streams are cast to bf16 at shard time (the tolerance for
this loss is 2e-2; bf16 keeps the scalar error ~1e-4), halving HBM traffic
to ~9.6MB/core. One-hot labels are exact in bf16, per-sample sums
accumulate in f32 on the ACT engine.

Schedule: every load rides the single SWDGE queue FIFO (two queues running
concurrently drop aggregate DMA from ~430 to ~320 GB/s, so one queue only):
labels (split so FIND0 unblocks early) -> row gathers as the index chain
delivers offsets -> x chunks, with the last tile split into quarters so the
sub/square tail after the final chunk is short. Per-core output is a
[128, 11] tile of per-sample (partial) sums; the host merges the last
tile's quarters, applies the clip (never binding for this data, but exact)
and takes the mean.
"""

import numpy as np
import ml_dtypes

import concourse.bacc as bacc
import concourse.bass as bass
import concourse.mybir as mybir
from concourse.tile import TileContext
from concourse.bass_utils import run_bass_kernel_spmd

F32 = mybir.dt.float32
BF16 = mybir.dt.bfloat16
U32 = mybir.dt.uint32
NP_BF16 = ml_dtypes.bfloat16

NCORES = 8
B = 8192          # full batch
C = 751           # num classes
D = 2048          # feature dim
BS = B // NCORES  # batch per core = 1024
P = 128           # partitions
NT = BS // P      # batch tiles per core = 8
NQ = 4            # last tile split into quarters for a short tail
NACC = NT - 1 + NQ

CLIP_LO, CLIP_HI = 1e-12, 1e12


def build_nc():
    nc = bacc.Bacc(
        "TRN2",
        target_bir_lowering=False,
        debug=False,
        num_devices=NCORES,
    )
    x = nc.dram_tensor("x", [BS, D], BF16, kind="ExternalInput")
    labels = nc.dram_tensor("labels", [BS, C], BF16, kind="ExternalInput")
    centers = nc.dram_tensor("centers", [C, D], BF16, kind="ExternalInput")
    out = nc.dram_tensor("out", [P, NACC], F32, kind="ExternalOutput")

    with TileContext(nc) as tc:
        with tc.tile_pool(name="big", bufs=1) as pool:
            ones = pool.tile([P, 8], BF16)
            idxs = pool.tile([P, NT, 8], U32)
            acc = pool.tile([P, NACC], F32)
            dif_a = pool.tile([P, D], BF16)
            dif_b = pool.tile([P, D], BF16)
            dsq = pool.tile([P, D], BF16)
            dsq2 = pool.tile([P, D], BF16)
            tjunk = pool.tile([P, D], BF16)
            lbig = pool.tile([P, NT, C], BF16)
            xbig = pool.tile([P, NT, D], BF16)
            ctile = pool.tile([P, NT, D], BF16)

            nc.vector.memset(ones[:], 1.0)

            labels_r = labels.rearrange("(n p) c -> p n c", p=P)
            x_r = x.rearrange("(n p) d -> p n d", p=P)

            # labels first on the SWDGE queue; first chunk is one tile so
            # FIND0 (and the first gather) unblocks early
            for lo, hi in ((0, 1), (1, 4), (4, 8)):
                nc.gpsimd.dma_start(
                    out=lbig[:, lo:hi, :], in_=labels_r[:, lo:hi, :]
                )

            # FIND_n + gather_n interleaved, with x chunks emitted between
            # gathers so the queue always has ready work while the index
            # chain paces the gathers; final x tile in quarters for a short
            # tail
            Q = D // NQ
            x_chunks = [("full", lo, hi) for lo, hi in ((0, 2), (2, 4), (4, 6), (6, 7))]
            x_chunks += [("quarter", q * Q, (q + 1) * Q) for q in range(NQ)]

            def load_x(i):
                kind, lo, hi = x_chunks[i]
                if kind == "full":
                    nc.gpsimd.dma_start(
                        out=xbig[:, lo:hi, :], in_=x_r[:, lo:hi, :]
                    )
                else:
                    nc.gpsimd.dma_start(
                        out=xbig[:, NT - 1, lo:hi], in_=x_r[:, NT - 1, lo:hi]
                    )

            for n in range(NT):
                nc.vector.max_index(
                    out=idxs[:, n, :], in_max=ones[:], in_values=lbig[:, n, :]
                )
                nc.gpsimd.indirect_dma_start(
                    out=ctile[:, n, :],
                    out_offset=None,
                    in_=centers[:],
                    in_offset=bass.IndirectOffsetOnAxis(
                        ap=idxs[:, n, 0:1], axis=0
                    ),
                )
                if n < len(x_chunks):
                    load_x(n)
            for i in range(NT, len(x_chunks)):
                load_x(i)

            # sub on DVE, then square + f32 accum spread across THREE
            # engines: ACT is capped at 1 elem/cycle (~2.6us/tile with the
            # accumulator read), so one tile + one quarter square on DVE
            # (mult + reduce_sum, ~3.5us) and one of each on the Pool
            # engine, which is idle once the DMA issues are done.
            SQ_ENGINE_FULL = {2: "dve", 5: "dve"}
            SQ_ENGINE_QUARTER = {1: "dve"}
            POOL_SUB_FULL = {1, 3}

            def square_accum(dif_ap, width, acc_col, eng):
                if eng == "act":
                    nc.scalar.activation(
                        out=dsq[:, 0:width],
                        in_=dif_ap,
                        func=mybir.ActivationFunctionType.Square,
                        accum_out=acc[:, acc_col:acc_col + 1],
                    )
                else:
                    nc.vector.tensor_mul(
                        out=tjunk[:, 0:width], in0=dif_ap, in1=dif_ap
                    )
                    nc.vector.reduce_sum(
                        out=acc[:, acc_col:acc_col + 1],
                        in_=tjunk[:, 0:width],
                        axis=mybir.AxisListType.X,
                    )

            for n in range(NT - 1):
                dif = dif_a if n % 2 == 0 else dif_b
                sub_eng = nc.gpsimd if n in POOL_SUB_FULL else nc.vector
                sub_eng.tensor_sub(
                    out=dif[:], in0=xbig[:, n, :], in1=ctile[:, n, :]
                )
                square_accum(dif[:], D, n, SQ_ENGINE_FULL.get(n, "act"))
            for q in range(NQ):
                sl = slice(q * Q, (q + 1) * Q)
                dif = dif_b if q % 2 == 0 else dif_a
                nc.vector.tensor_sub(
                    out=dif[:, sl], in0=xbig[:, NT - 1, sl], in1=ctile[:, NT - 1, sl]
                )
                square_accum(
                    dif[:, sl], Q, NT - 1 + q, SQ_ENGINE_QUARTER.get(q, "act")
                )

            # out store on the warm SWDGE queue
            nc.gpsimd.dma_start(out=out[:], in_=acc[:])

    nc.compile()
    return nc


_NC = None


def _get_nc():
    global _NC
    if _NC is None:
        _NC = build_nc()
    return _NC


def _shard(inputs: dict):
    x = np.asarray(inputs["x"]).astype(NP_BF16)
    labels = np.asarray(inputs["labels"]).astype(NP_BF16)
    centers = np.ascontiguousarray(np.asarray(inputs["centers"]).astype(NP_BF16))
    assert x.shape == (B, D) and labels.shape == (B, C) and centers.shape == (C, D)
    return [
        {
            "x": np.ascontiguousarray(x[k * BS:(k + 1) * BS]),
            "labels": np.ascontiguousarray(labels[k * BS:(k + 1) * BS]),
            "centers": centers,
        }
        for k in range(NCORES)
    ]


def run_sharded(inputs: dict, trace: bool = False):
    """Shard, run on 8 cores, return (per_sample [B] f32, BassKernelResults)."""
    in_maps = _shard(inputs)
    res = run_bass_kernel_spmd(
        _get_nc(), in_maps, core_ids=list(range(NCORES)), trace=trace
    )
    # out[p, n] holds sample k*BS + n*P + p; cols NT-1.. are the NQ
    # quarter-sums of the last tile
    def merge(o):
        last = o[:, NT - 1:].sum(axis=1, keepdims=True)
        return np.concatenate([o[:, :NT - 1], last], axis=1)

    per_sample = np.concatenate(
        [merge(res.results[k]["out"]).T.reshape(-1) for k in range(NCORES)]
    )
    return per_sample, res


def kernel(x, labels, centers):
    per_sample, _ = run_sharded({"x": x, "labels": labels, "centers": centers})
    per_sample = np.clip(per_sample, CLIP_LO, CLIP_HI)
    return np.asarray(per_sample.mean(dtype=np.float64), dtype=np.float32)


# revision 32
# speedup vs baseline: 1.2343x; 1.2343x over previous
"""CenterLoss kernel for Trainium2 (8 NeuronCores, data-parallel over batch).

reference:  mean(clip(rowsum((x - labels @ centers)^2), 1e-12, 1e12))
labels are exact one-hot rows, so labels @ centers is an embedding gather:
    idx[b]  = max_index(labels[b, :])           (DVE max_index, query = 1.0)
    c[b]    = centers[idx[b], :]                (indirect DMA row gather)
    ps[b]   = rowsum((x[b] - c[b])^2)           (DVE sub, ACT square+f32 accum)

All three input streams are cast to bf16 at shard time (the tolerance for
this loss is 2e-2; bf16 keeps the scalar error ~1e-4), halving HBM traffic
to ~9.6MB/core. One-hot labels are exact in bf16, per-sample sums
accumulate in f32 on the ACT engine.

Schedule: every load rides the single SWDGE queue FIFO (two queues running
concurrently drop aggregate DMA from ~430 to ~320 GB/s, so one queue only):
labels (split so FIND0 unblocks early) -> row gathers as the index chain
delivers offsets -> x chunks, with the last tile split into quarters so the
sub/square tail after the final chunk is short. Per-core output is a
[128, 11] tile of per-sample (partial) sums; the host merges the last
tile's quarters, applies the clip (never binding for this data, but exact)
and takes the mean.
"""

import numpy as np
import ml_dtypes

import concourse.bacc as bacc
import concourse.bass as bass
import concourse.mybir as mybir
from concourse.tile import TileContext
from concourse.bass_utils import run_bass_kernel_spmd

F32 = mybir.dt.float32
BF16 = mybir.dt.bfloat16
U32 = mybir.dt.uint32
NP_BF16 = ml_dtypes.bfloat16

NCORES = 8
B = 8192          # full batch
C = 751           # num classes
D = 2048          # feature dim
BS = B // NCORES  # batch per core = 1024
P = 128           # partitions
NT = BS // P      # batch tiles per core = 8
NQ = 4            # last tile split into quarters for a short tail
NACC = NT - 1 + NQ

CLIP_LO, CLIP_HI = 1e-12, 1e12


def build_nc():
    nc = bacc.Bacc(
        "TRN2",
        target_bir_lowering=False,
        debug=False,
        num_devices=NCORES,
    )
    x = nc.dram_tensor("x", [BS, D], BF16, kind="ExternalInput")
    labels = nc.dram_tensor("labels", [BS, C], BF16, kind="ExternalInput")
    centers = nc.dram_tensor("centers", [C, D], BF16, kind="ExternalInput")
    out = nc.dram_tensor("out", [P, NACC], F32, kind="ExternalOutput")

    with TileContext(nc) as tc:
        with tc.tile_pool(name="big", bufs=1) as pool:
            ones = pool.tile([P, 8], BF16)
            idxs = pool.tile([P, NT, 8], U32)
            acc = pool.tile([P, NACC], F32)
            dif_a = pool.tile([P, D], BF16)
            dif_b = pool.tile([P, D], BF16)
            dsq = pool.tile([P, D], BF16)
            dsq2 = pool.tile([P, D], BF16)
            tjunk = pool.tile([P, D], BF16)
            lbig = pool.tile([P, NT, C], BF16)
            xbig = pool.tile([P, NT, D], BF16)
            ctile = pool.tile([P, NT, D], BF16)

            nc.vector.memset(ones[:], 1.0)

            labels_r = labels.rearrange("(n p) c -> p n c", p=P)
            x_r = x.rearrange("(n p) d -> p n d", p=P)

            # labels first on the SWDGE queue; first chunk is one tile so
            # FIND0 (and the first gather) unblocks early
            for lo, hi in ((0, 1), (1, 4), (4, 8)):
                nc.gpsimd.dma_start(
                    out=lbig[:, lo:hi, :], in_=labels_r[:, lo:hi, :]
                )

            # FIND_n + gather_n interleaved, with x chunks emitted between
            # gathers so the queue always has ready work while the index
            # chain paces the gathers; final x tile in quarters for a short
            # tail
            Q = D // NQ
            x_chunks = [("full", lo, hi) for lo, hi in ((0, 2), (2, 4), (4, 6), (6, 7))]
            x_chunks += [("quarter", q * Q, (q + 1) * Q) for q in range(NQ)]

            def load_x(i):
                kind, lo, hi = x_chunks[i]
                if kind == "full":
                    nc.gpsimd.dma_start(
                        out=xbig[:, lo:hi, :], in_=x_r[:, lo:hi, :]
                    )
                else:
                    nc.gpsimd.dma_start(
                        out=xbig[:, NT - 1, lo:hi], in_=x_r[:, NT - 1, lo:hi]
                    )

            for n in range(NT):
                nc.vector.max_index(
                    out=idxs[:, n, :], in_max=ones[:], in_values=lbig[:, n, :]
                )
                nc.gpsimd.indirect_dma_start(
                    out=ctile[:, n, :],
                    out_offset=None,
                    in_=centers[:],
                    in_offset=bass.IndirectOffsetOnAxis(
                        ap=idxs[:, n, 0:1], axis=0
                    ),
                )
                if n < len(x_chunks):
                    load_x(n)
            for i in range(NT, len(x_chunks)):
                load_x(i)

            # sub on DVE, then square + f32 accum spread across THREE
            # engines: ACT is capped at 1 elem/cycle (~2.6us/tile with the
            # accumulator read), so one tile + one quarter square on DVE
            # (mult + reduce_sum, ~3.5us) and one of each on the Pool
            # engine, which is idle once the DMA issues are done.
            SQ_ENGINE_FULL = {2: "dve", 5: "dve"}
            SQ_ENGINE_QUARTER = {1: "dve"}
            POOL_SUB_FULL = set()

            def square_accum(dif_ap, width, acc_col, eng):
                if eng == "act":
                    nc.scalar.activation(
                        out=dsq[:, 0:width],
                        in_=dif_ap,
                        func=mybir.ActivationFunctionType.Square,
                        accum_out=acc[:, acc_col:acc_col + 1],
                    )
                else:
                    nc.vector.tensor_mul(
                        out=tjunk[:, 0:width], in0=dif_ap, in1=dif_ap
                    )
                    nc.vector.reduce_sum(
                        out=acc[:, acc_col:acc_col + 1],
                        in_=tjunk[:, 0:width],
                        axis=mybir.AxisListType.X,
                    )

            for n in range(NT - 1):
                dif = dif_a if n % 2 == 0 else dif_b
                sub_eng = nc.gpsimd if n in POOL_SUB_FULL else nc.vector
                sub_eng.tensor_sub(
                    out=dif[:], in0=xbig[:, n, :], in1=ctile[:, n, :]
                )
                square_accum(dif[:], D, n, SQ_ENGINE_FULL.get(n, "act"))
            for q in range(NQ):
                sl = slice(q * Q, (q + 1) * Q)
                dif = dif_b if q % 2 == 0 else dif_a
                nc.vector.tensor_sub(
                    out=dif[:, sl], in0=xbig[:, NT - 1, sl], in1=ctile[:, NT - 1, sl]
                )
                square_accum(
                    dif[:, sl], Q, NT - 1 + q, SQ_ENGINE_QUARTER.get(q, "act")
                )

            # out store on the warm SWDGE queue
            nc.gpsimd.dma_start(out=out[:], in_=acc[:])

    nc.compile()
    return nc


_NC = None


def _get_nc():
    global _NC
    if _NC is None:
        _NC = build_nc()
    return _NC


def _shard(inputs: dict):
    x = np.asarray(inputs["x"]).astype(NP_BF16)
    labels = np.asarray(inputs["labels"]).astype(NP_BF16)
    centers = np.ascontiguousarray(np.asarray(inputs["centers"]).astype(NP_BF16))
    assert x.shape == (B, D) and labels.shape == (B, C) and centers.shape == (C, D)
    return [
        {
            "x": np.ascontiguousarray(x[k * BS:(k + 1) * BS]),
            "labels": np.ascontiguousarray(labels[k * BS:(k + 1) * BS]),
            "centers": centers,
        }
        for k in range(NCORES)
    ]


def run_sharded(inputs: dict, trace: bool = False):
    """Shard, run on 8 cores, return (per_sample [B] f32, BassKernelResults)."""
    in_maps = _shard(inputs)
    res = run_bass_kernel_spmd(
        _get_nc(), in_maps, core_ids=list(range(NCORES)), trace=trace
    )
    # out[p, n] holds sample k*BS + n*P + p; cols NT-1.. are the NQ
    # quarter-sums of the last tile
    def merge(o):
        last = o[:, NT - 1:].sum(axis=1, keepdims=True)
        return np.concatenate([o[:, :NT - 1], last], axis=1)

    per_sample = np.concatenate(
        [merge(res.results[k]["out"]).T.reshape(-1) for k in range(NCORES)]
    )
    return per_sample, res


def kernel(x, labels, centers):
    per_sample, _ = run_sharded({"x": x, "labels": labels, "centers": centers})
    per_sample = np.clip(per_sample, CLIP_LO, CLIP_HI)
    return np.asarray(per_sample.mean(dtype=np.float64), dtype=np.float32)


# revision 34
# speedup vs baseline: 1.3743x; 1.1134x over previous
"""CenterLoss kernel for Trainium2 (8 NeuronCores, data-parallel over batch).

reference:  mean(clip(rowsum((x - labels @ centers)^2), 1e-12, 1e12))
labels are exact one-hot rows, so labels @ centers is an embedding gather:
    idx[b]  = max_index(labels[b, :])           (DVE max_index, query = 1.0)
    c[b]    = centers[idx[b], :]                (indirect DMA row gather)
    ps[b]   = rowsum((x[b] - c[b])^2)           (DVE sub, ACT square+f32 accum)

All three input streams are cast to bf16 at shard time (the tolerance for
this loss is 2e-2; bf16 keeps the scalar error ~1e-4), halving HBM traffic
to ~9.6MB/core. One-hot labels are exact in bf16, per-sample sums
accumulate in f32 on the ACT engine.

Schedule: every load rides the single SWDGE queue FIFO (two queues running
concurrently drop aggregate DMA from ~430 to ~320 GB/s, so one queue only):
labels (split so FIND0 unblocks early) -> row gathers as the index chain
delivers offsets -> x chunks, with the last tile split into quarters so the
sub/square tail after the final chunk is short. Per-core output is a
[128, 11] tile of per-sample (partial) sums; the host merges the last
tile's quarters, applies the clip (never binding for this data, but exact)
and takes the mean.
"""

import numpy as np
import ml_dtypes

import concourse.bacc as bacc
import concourse.bass as bass
import concourse.mybir as mybir
from concourse.tile import TileContext
from concourse.bass_utils import run_bass_kernel_spmd

F32 = mybir.dt.float32
BF16 = mybir.dt.bfloat16
U32 = mybir.dt.uint32
NP_BF16 = ml_dtypes.bfloat16

NCORES = 8
B = 8192          # full batch
C = 751           # num classes
D = 2048          # feature dim
BS = B // NCORES  # batch per core = 1024
P = 128           # partitions
NT = BS // P      # batch tiles per core = 8
NQ = 4            # last tile split into quarters for a short tail
NACC = NT - 1 + NQ

CLIP_LO, CLIP_HI = 1e-12, 1e12


def build_nc():
    nc = bacc.Bacc(
        "TRN2",
        target_bir_lowering=False,
        debug=False,
        num_devices=NCORES,
    )
    x = nc.dram_tensor("x", [BS, D], BF16, kind="ExternalInput")
    labels = nc.dram_tensor("labels", [BS, C], BF16, kind="ExternalInput")
    centers = nc.dram_tensor("centers", [C, D], BF16, kind="ExternalInput")
    out = nc.dram_tensor("out", [P, NACC], F32, kind="ExternalOutput")

    with TileContext(nc) as tc:
        with tc.tile_pool(name="big", bufs=1) as pool:
            ones = pool.tile([P, 8], BF16)
            idxs = pool.tile([P, NT, 8], U32)
            acc = pool.tile([P, NACC], F32)
            dif_a = pool.tile([P, D], BF16)
            dif_b = pool.tile([P, D], BF16)
            dsq = pool.tile([P, D], BF16)
            dsq2 = pool.tile([P, D], BF16)
            tjunk = pool.tile([P, D], BF16)
            lbig = pool.tile([P, NT, C], BF16)
            xbig = pool.tile([P, NT, D], BF16)
            ctile = pool.tile([P, NT, D], BF16)

            nc.vector.memset(ones[:], 1.0)

            labels_r = labels.rearrange("(n p) c -> p n c", p=P)
            x_r = x.rearrange("(n p) d -> p n d", p=P)

            # labels first; the single-tile first chunk rides the sync HWDGE
            # ring (shorter completion latency, warms the ring for the out
            # store) so FIND0 and the first gather unblock early
            nc.sync.dma_start(out=lbig[:, 0:1, :], in_=labels_r[:, 0:1, :])
            for lo, hi in ((1, 4), (4, 8)):
                nc.gpsimd.dma_start(
                    out=lbig[:, lo:hi, :], in_=labels_r[:, lo:hi, :]
                )

            # FIND_n + gather_n interleaved, with x chunks emitted between
            # gathers so the queue always has ready work while the index
            # chain paces the gathers; final x tile in quarters for a short
            # tail
            Q = D // NQ
            x_chunks = [("full", lo, hi) for lo, hi in ((0, 2), (2, 4), (4, 6), (6, 7))]
            x_chunks += [("quarter", q * Q, (q + 1) * Q) for q in range(NQ)]

            def load_x(i):
                kind, lo, hi = x_chunks[i]
                if kind == "full":
                    nc.gpsimd.dma_start(
                        out=xbig[:, lo:hi, :], in_=x_r[:, lo:hi, :]
                    )
                else:
                    nc.gpsimd.dma_start(
                        out=xbig[:, NT - 1, lo:hi], in_=x_r[:, NT - 1, lo:hi]
                    )

            for n in range(NT):
                nc.vector.max_index(
                    out=idxs[:, n, :], in_max=ones[:], in_values=lbig[:, n, :]
                )
                nc.gpsimd.indirect_dma_start(
                    out=ctile[:, n, :],
                    out_offset=None,
                    in_=centers[:],
                    in_offset=bass.IndirectOffsetOnAxis(
                        ap=idxs[:, n, 0:1], axis=0
                    ),
                )
                if n < len(x_chunks):
                    load_x(n)
            for i in range(NT, len(x_chunks)):
                load_x(i)

            # sub on DVE, then square + f32 accum spread across THREE
            # engines: ACT is capped at 1 elem/cycle (~2.6us/tile with the
            # accumulator read), so one tile + one quarter square on DVE
            # (mult + reduce_sum, ~3.5us) and one of each on the Pool
            # engine, which is idle once the DMA issues are done.
            SQ_ENGINE_FULL = {1: "dve", 2: "dve"}
            SQ_ENGINE_QUARTER = {}
            POOL_SUB_FULL = set()

            def square_accum(dif_ap, width, acc_col, eng):
                if eng == "act":
                    nc.scalar.activation(
                        out=dsq[:, 0:width],
                        in_=dif_ap,
                        func=mybir.ActivationFunctionType.Square,
                        accum_out=acc[:, acc_col:acc_col + 1],
                    )
                else:
                    nc.vector.tensor_mul(
                        out=tjunk[:, 0:width], in0=dif_ap, in1=dif_ap
                    )
                    nc.vector.reduce_sum(
                        out=acc[:, acc_col:acc_col + 1],
                        in_=tjunk[:, 0:width],
                        axis=mybir.AxisListType.X,
                    )

            for n in range(NT - 1):
                dif = dif_a if n % 2 == 0 else dif_b
                sub_eng = nc.gpsimd if n in POOL_SUB_FULL else nc.vector
                sub_eng.tensor_sub(
                    out=dif[:], in0=xbig[:, n, :], in1=ctile[:, n, :]
                )
                square_accum(dif[:], D, n, SQ_ENGINE_FULL.get(n, "act"))
            for q in range(NQ):
                sl = slice(q * Q, (q + 1) * Q)
                dif = dif_b if q % 2 == 0 else dif_a
                nc.vector.tensor_sub(
                    out=dif[:, sl], in0=xbig[:, NT - 1, sl], in1=ctile[:, NT - 1, sl]
                )
                square_accum(
                    dif[:, sl], Q, NT - 1 + q, SQ_ENGINE_QUARTER.get(q, "act")
                )

            # out store on the warmed sync ring (the idle Sync engine issues
            # it as soon as the last accumulator lands; the SWDGE drain does
            # not have to wait for it)
            nc.sync.dma_start(out=out[:], in_=acc[:])

    nc.compile()
    return nc


_NC = None


def _get_nc():
    global _NC
    if _NC is None:
        _NC = build_nc()
    return _NC


def _shard(inputs: dict):
    x = np.asarray(inputs["x"]).astype(NP_BF16)
    labels = np.asarray(inputs["labels"]).astype(NP_BF16)
    centers = np.ascontiguousarray(np.asarray(inputs["centers"]).astype(NP_BF16))
    assert x.shape == (B, D) and labels.shape == (B, C) and centers.shape == (C, D)
    return [
        {
            "x": np.ascontiguousarray(x[k * BS:(k + 1) * BS]),
            "labels": np.ascontiguousarray(labels[k * BS:(k + 1) * BS]),
            "centers": centers,
        }
        for k in range(NCORES)
    ]


def run_sharded(inputs: dict, trace: bool = False):
    """Shard, run on 8 cores, return (per_sample [B] f32, BassKernelResults)."""
    in_maps = _shard(inputs)
    res = run_bass_kernel_spmd(
        _get_nc(), in_maps, core_ids=list(range(NCORES)), trace=trace
    )
    # out[p, n] holds sample k*BS + n*P + p; cols NT-1.. are the NQ
    # quarter-sums of the last tile
    def merge(o):
        last = o[:, NT - 1:].sum(axis=1, keepdims=True)
        return np.concatenate([o[:, :NT - 1], last], axis=1)

    per_sample = np.concatenate(
        [merge(res.results[k]["out"]).T.reshape(-1) for k in range(NCORES)]
    )
    return per_sample, res


def kernel(x, labels, centers):
    per_sample, _ = run_sharded({"x": x, "labels": labels, "centers": centers})
    per_sample = np.clip(per_sample, CLIP_LO, CLIP_HI)
    return np.asarray(per_sample.mean(dtype=np.float64), dtype=np.float32)
